# revision 15
# baseline (speedup 1.0000x reference)
"""Trainium2 Bass kernel for nn_AttentionHead (softmax over query axis).

Sharding: 8 cores = 4 batches x 2 halves. Core c handles batch c//2 and
row-parity h=c%2: local 128-row chunk lc <-> global chunk g=2*lc+h.

Host ships x already transposed and bf16-cast: xt[e_chunk, e, t] so the
kernel does zero PE transposes for x. Per core:
  - projections kT/vT [128 D, 1024 t] from xt chunks; vT -> v natural
    via 8 PE transposes
  - ship kT+v early: AllGather across the pair overlaps q-projection
    and local-parity score blocks
  - scores sT[s, t] = kb.T @ qT, exp (scale 1/sqrt(128)) with per-key
    column sums (softmax normalizer is over the QUERY axis), causal mask
    via host-supplied mask tiles (h=0: [tri, zeros], h=1: [ones, tri]);
    local-parity key blocks first (no collective dependency)
  - AllReduce the [128,16] normalizer partials across the pair
  - z[t, :] = sum_s E[s,t] * (v[s,:]/Z[s]); single batched output DMA
Host assembles the 8 core outputs back into [4, 2048, 128].
"""
import sys

for _p in ("/opt/trn_rl_repo",):
    if _p not in sys.path:
        sys.path.append(_p)

import numpy as np
import ml_dtypes

import concourse.bass as bass
import concourse.mybir as mybir
import concourse.tile as tile
from concourse import bacc
from concourse.bass import ds, ts
from concourse.bass_utils import run_bass_kernel_spmd
from concourse.masks import make_identity

BF16 = mybir.dt.bfloat16
F32 = mybir.dt.float32
AF = mybir.ActivationFunctionType
ALU = mybir.AluOpType
AX = mybir.AxisListType

B, T, E, D = 4, 2048, 2048, 128
NLC = 8          # local 128-row chunks per core
NE = 16          # E chunks of 128
NSB = 16         # key blocks of 128
TLOC = NLC * 128
SCALE = 1.0 / np.sqrt(D)
N_CORES = 8
REPLICA_GROUPS = [[0, 1], [2, 3], [4, 5], [6, 7]]


def build_nc():
    nc = bacc.Bacc("TRN2", target_bir_lowering=False, debug=False,
                   num_devices=N_CORES)
    xt = nc.dram_tensor("xt", [NE, 128, TLOC], BF16, kind="ExternalInput")
    wq = nc.dram_tensor("wq", [128, NE, D], BF16, kind="ExternalInput")
    wk = nc.dram_tensor("wk", [128, NE, D], BF16, kind="ExternalInput")
    wv = nc.dram_tensor("wv", [128, NE, D], BF16, kind="ExternalInput")
    masks = nc.dram_tensor("masks", [128, 2, 128], BF16, kind="ExternalInput")
    out = nc.dram_tensor("out", [TLOC, D], F32, kind="ExternalOutput")

    with tile.TileContext(nc) as tc:
        _body(nc, tc, xt, wq, wk, wv, masks, out)
    nc.compile()
    return nc


def _body(nc, tc, xt, wq, wk, wv, masks, out):
    # parity h of this core is encoded in the host-built mask tiles; the
    # kernel program itself is parity-independent.
    with (
        tc.tile_pool(name="const", bufs=1) as const_pool,
        tc.tile_pool(name="dram", bufs=1, space="DRAM") as dram_pool,
        tc.tile_pool(name="proj", bufs=1) as proj_pool,
        tc.tile_pool(name="escore", bufs=1) as e_pool,
    ):
        # ---- constants (gpsimd SWDGE path; sync ring stays free for xt) ----
        ident = const_pool.tile([128, 128], BF16, name="ident")
        make_identity(nc, ident)
        wq_sb = const_pool.tile([128, NE, D], BF16, name="wq_sb")
        wk_sb = const_pool.tile([128, NE, D], BF16, name="wk_sb")
        wv_sb = const_pool.tile([128, NE, D], BF16, name="wv_sb")
        nc.gpsimd.dma_start(out=wk_sb[:], in_=wk[:])
        nc.gpsimd.dma_start(out=wv_sb[:], in_=wv[:])
        nc.gpsimd.dma_start(out=wq_sb[:], in_=wq[:])
        masks_sb = const_pool.tile([128, 2, 128], BF16, name="masks_sb")
        nc.gpsimd.dma_start(out=masks_sb[:], in_=masks[:])

        # ---- xT load: one contiguous DMA per e-chunk ----
        xt_sb = const_pool.tile([128, NE, TLOC], BF16, name="xt_sb")
        for c in range(NE):
            nc.sync.dma_start(out=xt_sb[:, c, :], in_=xt[c])

        kT_loc = proj_pool.tile([128, TLOC], BF16, name="kT_loc")
        vT_sb = proj_pool.tile([128, TLOC], BF16, name="vT_sb")
        v_loc = proj_pool.tile([128, TLOC], BF16, name="v_loc")
        qT_sb = proj_pool.tile([128, TLOC], BF16, name="qT_sb")

        # bar1 carries kT; bar1.5 carries v; bar2 is the zsum AllReduce.
        cc1_in = dram_pool.tile([128, TLOC], BF16, name="cc1_in")
        cc1_out = dram_pool.tile([2, 128, TLOC], BF16, name="cc1_out")
        cc2_in = dram_pool.tile([128, TLOC], BF16, name="cc2_in")
        cc2_out = dram_pool.tile([2, 128, TLOC], BF16, name="cc2_out")
        zin = [dram_pool.tile([128, NSB // 2], F32, name=f"zin{i}")
               for i in range(2)]
        zout = [dram_pool.tile([128, NSB // 2], F32, name=f"zout_d{i}")
                for i in range(2)]

        # PE warmup spin: get HAM to K=8/8 while the xt DMA streams in.
        zeros = const_pool.tile([128, 128], BF16, name="zeros")
        nc.vector.memset(zeros[:], 0.0)
        with tc.tile_pool(name="wu_psum", bufs=1, space="PSUM") as wu_psum:
            wu = wu_psum.tile([128, 128], F32, tag="wu")
            for _ in range(44):
                nc.tensor.matmul(wu[:], lhsT=zeros[:], rhs=zeros[:],
                                 start=True, stop=True)

        with (
            tc.tile_pool(name="pj_psum", bufs=2, space="PSUM") as pj_psum,
            tc.tile_pool(name="tp_psum", bufs=2, space="PSUM") as tp_psum,
        ):
            # ---- k projection first; stage each piece, then trigger bar1 ----
            for piece in range(2):
                k_ps = pj_psum.tile([128, 512], F32, tag="k_ps")
                for e in range(NE):
                    nc.tensor.matmul(
                        k_ps[:], lhsT=wk_sb[:, e, :],
                        rhs=xt_sb[:, e, ts(piece, 512)],
                        start=(e == 0), stop=(e == NE - 1),
                    )
                nc.scalar.copy(out=kT_loc[:, ts(piece, 512)], in_=k_ps[:])
                nc.sync.dma_start(out=cc1_in[:, ts(piece, 512)],
                                  in_=kT_loc[:, ts(piece, 512)])
            nc.gpsimd.collective_compute(
                "AllGather", ALU.bypass, replica_groups=REPLICA_GROUPS,
                ins=[cc1_in[:].opt()], outs=[cc1_out[:].opt()],
            )

            # ---- q, v projections + v transpose run in bar1's shadow ----
            for piece in range(2):
                q_ps = pj_psum.tile([128, 512], F32, tag="q_ps")
                for e in range(NE):
                    nc.tensor.matmul(
                        q_ps[:], lhsT=wq_sb[:, e, :],
                        rhs=xt_sb[:, e, ts(piece, 512)],
                        start=(e == 0), stop=(e == NE - 1),
                    )
                nc.scalar.copy(out=qT_sb[:, ts(piece, 512)], in_=q_ps[:])
            for piece in range(2):
                vt_ps = pj_psum.tile([128, 512], F32, tag="vt_ps")
                for e in range(NE):
                    nc.tensor.matmul(
                        vt_ps[:], lhsT=wv_sb[:, e, :],
                        rhs=xt_sb[:, e, ts(piece, 512)],
                        start=(e == 0), stop=(e == NE - 1),
                    )
                nc.vector.tensor_copy(out=vT_sb[:, ts(piece, 512)], in_=vt_ps[:])
            for half in range(2):
                tpv = tp_psum.tile([128, 512], BF16, tag="tp")
                for j in range(4):
                    lc = half * 4 + j
                    nc.tensor.transpose(
                        out=tpv[:, ts(j, 128)],
                        in_=vT_sb[:, ts(lc, 128)],
                        identity=ident[:],
                    )
                nc.vector.tensor_copy(out=v_loc[:, ts(half, 512)], in_=tpv[:])
                nc.sync.dma_start(out=cc2_in[:, ds(half * 512, 512)],
                                  in_=v_loc[:, ts(half, 512)])
            nc.gpsimd.collective_compute(
                "AllGather", ALU.bypass, replica_groups=REPLICA_GROUPS,
                ins=[cc2_in[:].opt()], outs=[cc2_out[:].opt()],
            )

        # kT halves by rank (own rank's half is bit-identical to kT_loc;
        # reading both keeps the program parity-independent).
        kT_all = proj_pool.tile([128, 2, TLOC], BF16, name="kT_all")
        nc.sync.dma_start(out=kT_all[:],
                          in_=cc1_out[:].rearrange("r p t -> p r t"))
        v_all = proj_pool.tile([128, 2, TLOC], BF16, name="v_all")
        nc.sync.dma_start(out=v_all[:],
                          in_=cc2_out[:].rearrange("r p t -> p r t"))

        # ---- scores / exp / normalizer partials ----
        stats = const_pool.tile([128, NSB * 4], F32, name="stats")
        zsum_loc = const_pool.tile([128, NSB], F32, name="zsum_loc")
        nc.vector.memset(stats[:], 0.0)
        e_tiles = {}  # (sb, lc) -> AP [128 s, 128 t]
        with tc.tile_pool(name="sc_psum", bufs=3, space="PSUM") as sc_psum:
            # diagonal-straddling blocks, batched 4 per exp
            for grp in range(4):
                dg = sc_psum.tile([128, 512], F32, tag="dgm")
                for j in range(4):
                    sb = grp * 4 + j
                    lo = sb // 2
                    nc.tensor.matmul(
                        dg[:, ts(j, 128)], lhsT=kT_all[:, sb % 2, ds(lo * 128, 128)],
                        rhs=qT_sb[:, ds(lo * 128, 128)], start=True, stop=True)
                em4 = e_pool.tile([128, 512], BF16, name=f"em4_{grp}",
                                  tag=f"em4_{grp}")
                nc.scalar.activation(out=em4[:], in_=dg[:], func=AF.Exp,
                                     scale=SCALE)
                for half in range(2):
                    nc.vector.tensor_tensor(
                        out=em4[:, ts(half, 256)], in0=em4[:, ts(half, 256)],
                        in1=masks_sb[:].rearrange("p a b -> p (a b)"),
                        op=ALU.mult)
                for j in range(4):
                    sb = grp * 4 + j
                    nc.vector.reduce_sum(out=stats[:, ds(sb * 4 + 3, 1)],
                                         in_=em4[:, ts(j, 128)], axis=AX.X)
                    e_tiles[(sb, sb // 2)] = em4[:, ts(j, 128)]
            # full blocks per key chunk, with accumulated column sums
            for sb in range(NSB):
                lo = sb // 2
                kb = kT_all[:, sb % 2, ds(lo * 128, 128)]
                start_lc = lo + 1
                pidx = 0
                while start_lc < NLC:
                    n = min(4, NLC - start_lc)
                    scf = sc_psum.tile([128, 512], F32, tag="scf")
                    nc.tensor.matmul(
                        scf[:, ds(0, n * 128)], lhsT=kb,
                        rhs=qT_sb[:, ds(start_lc * 128, n * 128)],
                        start=True, stop=True,
                    )
                    ef = e_pool.tile([128, n * 128], BF16,
                                     name=f"ef{sb}_{pidx}", tag=f"ef{sb}_{pidx}")
                    nc.scalar.activation(
                        out=ef[:], in_=scf[:, ds(0, n * 128)], func=AF.Exp,
                        scale=SCALE, accum_out=stats[:, ds(sb * 4 + pidx, 1)],
                    )
                    for j in range(n):
                        e_tiles[(sb, start_lc + j)] = ef[:, ts(j, 128)]
                    start_lc += n
                    pidx += 1
                nc.vector.reduce_sum(out=zsum_loc[:, ds(sb, 1)],
                                     in_=stats[:, ds(sb * 4, 4)], axis=AX.X)
                # chunked zsum exchange: ship sb 0-7 while 8-15 still compute
                if sb == 7 or sb == 15:
                    i = sb // 8
                    nc.sync.dma_start(out=zin[i][:],
                                      in_=zsum_loc[:, ds(i * 8, 8)])
                    nc.gpsimd.collective_compute(
                        "AllReduce", ALU.add, replica_groups=REPLICA_GROUPS,
                        ins=[zin[i][:].opt()], outs=[zout[i][:].opt()],
                    )

            # ---- reciprocal + v scaling, per zsum half ----
            zsum_full = const_pool.tile([128, NSB], F32, name="zsum_full")
            recip = const_pool.tile([128, NSB], F32, name="recip")
            v_scaled = [proj_pool.tile([128, 1024], BF16, name=f"v_scaled{i}")
                        for i in range(2)]
            for i in range(2):
                nc.sync.dma_start(out=zsum_full[:, ds(i * 8, 8)], in_=zout[i][:])
                nc.vector.reciprocal(out=recip[:, ds(i * 8, 8)],
                                     in_=zsum_full[:, ds(i * 8, 8)])
                for j in range(NSB // 2):
                    sb = i * 8 + j
                    nc.vector.tensor_scalar_mul(
                        out=v_scaled[i][:, ds(j * 128, 128)],
                        in0=v_all[:, sb % 2, ds((sb // 2) * 128, 128)],
                        scalar1=recip[:, ds(sb, 1)],
                    )

            # ---- z = A @ v' per local chunk; batched output DMA ----
            z_all = const_pool.tile([128, NLC, D], F32, name="z_all")
            with tc.tile_pool(name="av_psum", bufs=2, space="PSUM") as av_psum:
                for lc in range(NLC):
                    zp = av_psum.tile([128, D], F32, tag="zp")
                    nsb = 2 * lc + 2
                    for sb in range(nsb):
                        nc.tensor.matmul(
                            zp[:], lhsT=e_tiles[(sb, lc)],
                            rhs=v_scaled[sb // 8][:, ds((sb % 8) * 128, 128)],
                            start=(sb == 0), stop=(sb == nsb - 1),
                        )
                    if lc % 2 == 0:
                        nc.vector.tensor_copy(out=z_all[:, lc, :], in_=zp[:])
                    else:
                        nc.scalar.copy(out=z_all[:, lc, :], in_=zp[:])
                    if lc == 3 or lc == 7:
                        nc.sync.dma_start(
                            out=out[ds((lc // 4) * 512, 512), :]
                                .rearrange("(c p) d -> p c d", p=128),
                            in_=z_all[:, ds((lc // 4) * 4, 4), :],
                        )


_NC_CACHE = None


def _get_nc():
    global _NC_CACHE
    if _NC_CACHE is None:
        _NC_CACHE = build_nc()
    return _NC_CACHE


def _host_masks(h: int) -> np.ndarray:
    tri = (np.arange(128)[None, :] >= np.arange(128)[:, None]).astype(np.float32)
    ones = np.ones((128, 128), np.float32)
    zeros = np.zeros((128, 128), np.float32)
    pair = [tri, zeros] if h == 0 else [ones, tri]
    return np.ascontiguousarray(np.stack(pair, axis=0).transpose(1, 0, 2))


def build_in_maps(x_in, Wq, Wk, Wv):
    """Host-side sharding: per-core transposed bf16 x + rearranged weights."""
    x_in = np.asarray(x_in, dtype=np.float32)
    ws = {}
    for name, W in (("wq", Wq), ("wk", Wk), ("wv", Wv)):
        W = np.asarray(W, dtype=np.float32)
        ws[name] = np.ascontiguousarray(
            W.reshape(NE, 128, D).transpose(1, 0, 2)
        ).astype(ml_dtypes.bfloat16)
    in_maps = []
    for c in range(N_CORES):
        b, h = c // 2, c % 2
        rows = np.concatenate(
            [x_in[b, (2 * lc + h) * 128:(2 * lc + h + 1) * 128]
             for lc in range(NLC)]
        )  # [1024, 2048] f32
        xt = np.ascontiguousarray(rows.T).reshape(NE, 128, TLOC)
        in_maps.append({
            "xt": xt.astype(ml_dtypes.bfloat16),
            "wq": ws["wq"], "wk": ws["wk"], "wv": ws["wv"],
            "masks": _host_masks(h).astype(ml_dtypes.bfloat16),
        })
    return in_maps


def kernel(x_in, Wq, Wk, Wv):
    nc = _get_nc()
    in_maps = build_in_maps(x_in, Wq, Wk, Wv)
    res = run_bass_kernel_spmd(nc, in_maps, core_ids=list(range(N_CORES)))
    out = np.empty((B, T, D), np.float32)
    for c in range(N_CORES):
        b, h = c // 2, c % 2
        o = res.results[c]["out"]
        for lc in range(NLC):
            g = 2 * lc + h
            out[b, g * 128:(g + 1) * 128] = o[lc * 128:(lc + 1) * 128]
    return out


# revision 16
# speedup vs baseline: 1.0213x; 1.0213x over previous
"""Trainium2 Bass kernel for nn_AttentionHead (softmax over query axis).

Zero-collective design: 8 cores, core pair (2b, 2b+1) both compute batch b
end-to-end (fully redundant); the host reads the even core's output. No
cross-core collectives -> no global-barrier rendezvous, no sensitivity to
the 10-35us per-core launch stagger.

Host ships x already transposed and bf16-cast (xt[e_chunk, e, t]) so the
kernel does zero PE transposes for x. Per core:
  - projections kT/qT/vT [128 D, 2048 t] over the full batch; vT -> v
    natural via 16 PE transposes
  - scores sT[s, t] = kb.T @ qT for the full causal triangle, exp with
    per-key column sums (softmax normalizer is over the QUERY axis);
    diagonal blocks batched 4-per-exp and masked with a host tri tile
  - Z is fully local (all queries present): reciprocal + v scaling
  - z[t, :] = sum_s E[s,t] * (v[s,:]/Z[s]); output streamed in quarters
"""
import sys

for _p in ("/opt/trn_rl_repo",):
    if _p not in sys.path:
        sys.path.append(_p)

import numpy as np
import ml_dtypes

import concourse.bass as bass
import concourse.mybir as mybir
import concourse.tile as tile
from concourse import bacc
from concourse.bass import ds, ts
from concourse.bass_utils import run_bass_kernel_spmd
from concourse.masks import make_identity

BF16 = mybir.dt.bfloat16
F32 = mybir.dt.float32
AF = mybir.ActivationFunctionType
ALU = mybir.AluOpType
AX = mybir.AxisListType

B, T, E, D = 4, 2048, 2048, 128
NE = 16          # E chunks of 128
NTC = 16         # t chunks of 128
NSB = 16         # key blocks of 128
SCALE = 1.0 / np.sqrt(D)
N_CORES = 8


def build_nc():
    nc = bacc.Bacc("TRN2", target_bir_lowering=False, debug=False,
                   num_devices=N_CORES)
    xt = nc.dram_tensor("xt", [NE, 128, T], BF16, kind="ExternalInput")
    wq = nc.dram_tensor("wq", [128, NE, D], BF16, kind="ExternalInput")
    wk = nc.dram_tensor("wk", [128, NE, D], BF16, kind="ExternalInput")
    wv = nc.dram_tensor("wv", [128, NE, D], BF16, kind="ExternalInput")
    masks = nc.dram_tensor("masks", [128, 512], BF16, kind="ExternalInput")
    out = nc.dram_tensor("out", [T, D], F32, kind="ExternalOutput")

    with tile.TileContext(nc) as tc:
        _body(nc, tc, xt, wq, wk, wv, masks, out)
    nc.compile()
    return nc


def _body(nc, tc, xt, wq, wk, wv, masks, out):
    with (
        tc.tile_pool(name="const", bufs=1) as const_pool,
        tc.tile_pool(name="proj", bufs=1) as proj_pool,
        tc.tile_pool(name="escore", bufs=1) as e_pool,
    ):
        # ---- constants (gpsimd SWDGE path; sync ring stays free for xt) ----
        ident = const_pool.tile([128, 128], BF16, name="ident")
        make_identity(nc, ident)
        wq_sb = const_pool.tile([128, NE, D], BF16, name="wq_sb")
        wk_sb = const_pool.tile([128, NE, D], BF16, name="wk_sb")
        wv_sb = const_pool.tile([128, NE, D], BF16, name="wv_sb")
        nc.gpsimd.dma_start(out=wk_sb[:], in_=wk[:])
        nc.gpsimd.dma_start(out=wq_sb[:], in_=wq[:])
        nc.gpsimd.dma_start(out=wv_sb[:], in_=wv[:])
        masks_sb = const_pool.tile([128, 512], BF16, name="masks_sb")
        nc.gpsimd.dma_start(out=masks_sb[:], in_=masks[:])

        # ---- xT load: one contiguous DMA per e-chunk ----
        xt_sb = const_pool.tile([128, NE, T], BF16, name="xt_sb")
        for c in range(NE):
            nc.sync.dma_start(out=xt_sb[:, c, :], in_=xt[c])

        kT_sb = proj_pool.tile([128, T], BF16, name="kT_sb")
        qT_sb = proj_pool.tile([128, T], BF16, name="qT_sb")
        vT_sb = proj_pool.tile([128, T], BF16, name="vT_sb")
        v_nat = proj_pool.tile([128, NSB, D], BF16, name="v_nat")

        # PE warmup spin: get HAM to K=8/8 while the xt DMA streams in.
        zeros = const_pool.tile([128, 128], BF16, name="zeros")
        nc.vector.memset(zeros[:], 0.0)
        with tc.tile_pool(name="wu_psum", bufs=1, space="PSUM") as wu_psum:
            wu = wu_psum.tile([128, 128], F32, tag="wu")
            for _ in range(44):
                nc.tensor.matmul(wu[:], lhsT=zeros[:], rhs=zeros[:],
                                 start=True, stop=True)

        with (
            tc.tile_pool(name="pj_psum", bufs=2, space="PSUM") as pj_psum,
            tc.tile_pool(name="tp_psum", bufs=2, space="PSUM") as tp_psum,
        ):
            for tag, w_sb, dst, eng in (
                ("k_ps", wk_sb, kT_sb, "scalar"),
                ("q_ps", wq_sb, qT_sb, "scalar"),
                ("vt_ps", wv_sb, vT_sb, "vector"),
            ):
                for piece in range(4):
                    ps = pj_psum.tile([128, 512], F32, tag=tag)
                    for e in range(NE):
                        nc.tensor.matmul(
                            ps[:], lhsT=w_sb[:, e, :],
                            rhs=xt_sb[:, e, ts(piece, 512)],
                            start=(e == 0), stop=(e == NE - 1),
                        )
                    if eng == "scalar":
                        nc.scalar.copy(out=dst[:, ts(piece, 512)], in_=ps[:])
                    else:
                        nc.vector.tensor_copy(out=dst[:, ts(piece, 512)],
                                              in_=ps[:])
            for grp in range(4):
                tpv = tp_psum.tile([128, 512], BF16, tag="tp")
                for j in range(4):
                    sc = grp * 4 + j
                    nc.tensor.transpose(
                        out=tpv[:, ts(j, 128)],
                        in_=vT_sb[:, ts(sc, 128)],
                        identity=ident[:],
                    )
                nc.vector.tensor_copy(out=v_nat[:, ds(grp * 4, 4), :]
                                      .rearrange("p c d -> p (c d)"),
                                      in_=tpv[:])

        # ---- scores / exp / normalizer (all local: full causal triangle) ----
        stats = const_pool.tile([128, NSB * 5], F32, name="stats")
        zsum_loc = const_pool.tile([128, NSB], F32, name="zsum_loc")
        nc.vector.memset(stats[:], 0.0)
        e_tiles = {}  # (sb, tc) -> AP [128 s, 128 t]
        with tc.tile_pool(name="sc_psum", bufs=3, space="PSUM") as sc_psum:
            # diagonal blocks, batched 4 per exp, tri-masked
            for grp in range(4):
                dg = sc_psum.tile([128, 512], F32, tag="dgm")
                for j in range(4):
                    sb = grp * 4 + j
                    nc.tensor.matmul(
                        dg[:, ts(j, 128)], lhsT=kT_sb[:, ds(sb * 128, 128)],
                        rhs=qT_sb[:, ds(sb * 128, 128)], start=True, stop=True)
                em4 = e_pool.tile([128, 512], BF16, name=f"em4_{grp}",
                                  tag=f"em4_{grp}")
                nc.scalar.activation(out=em4[:], in_=dg[:], func=AF.Exp,
                                     scale=SCALE)
                nc.vector.tensor_tensor(out=em4[:], in0=em4[:],
                                        in1=masks_sb[:], op=ALU.mult)
                for j in range(4):
                    sb = grp * 4 + j
                    nc.vector.reduce_sum(out=stats[:, ds(sb * 5 + 4, 1)],
                                         in_=em4[:, ts(j, 128)], axis=AX.X)
                    e_tiles[(sb, sb)] = em4[:, ts(j, 128)]
            # full blocks per key chunk, with accumulated column sums
            for sb in range(NSB):
                kb = kT_sb[:, ds(sb * 128, 128)]
                start_tc = sb + 1
                pidx = 0
                while start_tc < NTC:
                    n = min(4, NTC - start_tc)
                    scf = sc_psum.tile([128, 512], F32, tag="scf")
                    nc.tensor.matmul(
                        scf[:, ds(0, n * 128)], lhsT=kb,
                        rhs=qT_sb[:, ds(start_tc * 128, n * 128)],
                        start=True, stop=True,
                    )
                    ef = e_pool.tile([128, n * 128], BF16,
                                     name=f"ef{sb}_{pidx}", tag=f"ef{sb}_{pidx}")
                    nc.scalar.activation(
                        out=ef[:], in_=scf[:, ds(0, n * 128)], func=AF.Exp,
                        scale=SCALE, accum_out=stats[:, ds(sb * 5 + pidx, 1)],
                    )
                    for j in range(n):
                        e_tiles[(sb, start_tc + j)] = ef[:, ts(j, 128)]
                    start_tc += n
                    pidx += 1
                nc.vector.reduce_sum(out=zsum_loc[:, ds(sb, 1)],
                                     in_=stats[:, ds(sb * 5, 5)], axis=AX.X)

            # ---- reciprocal + v scaling (no exchange needed) ----
            recip = const_pool.tile([128, NSB], F32, name="recip")
            nc.vector.reciprocal(out=recip[:], in_=zsum_loc[:])
            v_scaled = proj_pool.tile([128, NSB, D], BF16, name="v_scaled")
            for sb in range(NSB):
                nc.vector.tensor_scalar_mul(
                    out=v_scaled[:, sb, :],
                    in0=v_nat[:, sb, :],
                    scalar1=recip[:, ds(sb, 1)],
                )

            # ---- z = A @ v' per t chunk; output streamed in quarters ----
            z_all = const_pool.tile([128, NTC, D], F32, name="z_all")
            with tc.tile_pool(name="av_psum", bufs=2, space="PSUM") as av_psum:
                for g in range(NTC):
                    zp = av_psum.tile([128, D], F32, tag="zp")
                    nsb = g + 1
                    for sb in range(nsb):
                        nc.tensor.matmul(
                            zp[:], lhsT=e_tiles[(sb, g)],
                            rhs=v_scaled[:, sb, :],
                            start=(sb == 0), stop=(sb == nsb - 1),
                        )
                    if g % 2 == 0:
                        nc.vector.tensor_copy(out=z_all[:, g, :], in_=zp[:])
                    else:
                        nc.scalar.copy(out=z_all[:, g, :], in_=zp[:])
                    if g % 4 == 3:
                        q4 = g // 4
                        nc.sync.dma_start(
                            out=out[ds(q4 * 512, 512), :]
                                .rearrange("(c p) d -> p c d", p=128),
                            in_=z_all[:, ds(q4 * 4, 4), :],
                        )


_NC_CACHE = None


def _get_nc():
    global _NC_CACHE
    if _NC_CACHE is None:
        _NC_CACHE = build_nc()
    return _NC_CACHE


def build_in_maps(x_in, Wq, Wk, Wv):
    """Host-side prep: full-batch transposed bf16 x; pair cores share inputs."""
    x_in = np.asarray(x_in, dtype=np.float32)
    ws = {}
    for name, W in (("wq", Wq), ("wk", Wk), ("wv", Wv)):
        W = np.asarray(W, dtype=np.float32)
        ws[name] = np.ascontiguousarray(
            W.reshape(NE, 128, D).transpose(1, 0, 2)
        ).astype(ml_dtypes.bfloat16)
    tri = (np.arange(128)[None, :] >= np.arange(128)[:, None]).astype(np.float32)
    masks = np.ascontiguousarray(np.tile(tri, (1, 4))).astype(ml_dtypes.bfloat16)
    per_batch = []
    for b in range(B):
        xt = np.ascontiguousarray(x_in[b].T).reshape(NE, 128, T)
        per_batch.append(xt.astype(ml_dtypes.bfloat16))
    in_maps = []
    for c in range(N_CORES):
        in_maps.append({
            "xt": per_batch[c // 2],
            "wq": ws["wq"], "wk": ws["wk"], "wv": ws["wv"],
            "masks": masks,
        })
    return in_maps


def kernel(x_in, Wq, Wk, Wv):
    nc = _get_nc()
    in_maps = build_in_maps(x_in, Wq, Wk, Wv)
    res = run_bass_kernel_spmd(nc, in_maps, core_ids=list(range(N_CORES)))
    out = np.empty((B, T, D), np.float32)
    for b in range(B):
        out[b] = res.results[2 * b]["out"]
    return out
